# revision 1
# baseline (speedup 1.0000x reference)
"""Kalman filter kernel for 8x Trainium2 NeuronCores.

Math: the covariance/gain recursion (P_t, K_t) is data-independent and, with
closed-loop spectral radius ~0.5, converges to its steady state within ~30
steps at float precision.  After convergence the state recursion is the LTI
scan  z_t = M z_{t-1} + NK @ [u_t; x_t]  with  M = (I-KC)A,  NK = [(I-KC)B, K].
Because ||M^d|| < 1e-15 for d >= 48, the scan equals a 64-tap causal FIR
filter, which we factor as three dilated 4-tap stages (strides 16/4/1):

    g(t) = sum_p (M^16p NK) v(t-16p)      p = 0..3   (contraction 128)
    h(t) = sum_q  M^4q      g(t-4q)       q = 0..3   (contraction 64)
    z(t) = sum_r  M^r       h(t-r)        r = 0..3   (contraction 64)

so z(t) = sum_{d<64} M^d NK v(t-d).  Each stage is a handful of PSUM-
accumulated matmuls over the core's 1024 output columns -> embarrassingly
parallel across the 8 cores (each takes a 64-column halo from its left
neighbour; no collectives).  The first 96 columns get an additive patch for
the time-varying transient (gains not yet converged), computed from the
first 96 input columns.
"""

import numpy as np

L = 64          # latent size
NV = 128        # stacked input dim [u; x]
T = 8192
NCORES = 8
TC = T // NCORES            # 1024 output columns per core
HALO = 64                   # FIR window
WIDTH = HALO + TC           # per-core input columns
NTAP = 4
NRIC = 64                   # Riccati iterations (converged far past f32 by then)
T0 = 96                     # transient patch columns

F32 = np.float32


# ----------------------------------------------------------------------------
# host-side parameter preprocessing (data-independent)
# ----------------------------------------------------------------------------

def _gains(A, B, C, Q, R):
    """float64 Riccati recursion -> per-step (M_t, NK_t) and steady (M, NK)."""
    Ad, Bd, Cd, Qd, Rd = (m.astype(np.float64) for m in (A, B, C, Q, R))
    P = np.eye(L)
    Ms, NKs = [], []
    for _ in range(NRIC):
        Pp = Ad @ P @ Ad.T + Qd
        S = Cd @ Pp @ Cd.T + Rd
        K = Pp @ Cd.T @ np.linalg.inv(S)
        P = Pp - K @ (Cd @ Pp)
        IKC = np.eye(L) - K @ Cd
        Ms.append(IKC @ Ad)
        NKs.append(np.concatenate([IKC @ Bd, K], axis=1))   # [L, NV]
    return Ms, NKs


def _mpow(M, k):
    out = np.eye(L)
    for _ in range(k):
        out = M @ out
    return out


def _taps(Mss, NKss):
    """FIR tap matrices for the three dilated stages (float32)."""
    t1 = np.stack([(_mpow(Mss, 16 * p) @ NKss) for p in range(NTAP)])  # [4,L,NV]
    t2 = np.stack([_mpow(Mss, 4 * q) for q in range(NTAP)])            # [4,L,L]
    t3 = np.stack([_mpow(Mss, r) for r in range(NTAP)])                # [4,L,L]
    return t1.astype(F32), t2.astype(F32), t3.astype(F32)


def _fir_host(vpad, t1, t2, t3, ncols):
    """Replicate the device FIR pipeline on host for the first `ncols`
    output columns.  vpad: [NV, 64 + ncols] (64 left-pad columns)."""
    n = vpad.shape[1]

    def dilated(taps, src, stride):
        out = np.zeros((L, n), F32)
        for i in range(NTAP):
            sh = i * stride
            out[:, sh:] += (taps[i] @ src[:, : n - sh]).astype(F32)
        return out

    g = dilated(t1, vpad, 16)
    h = dilated(t2, g, 4)
    z = dilated(t3, h, 1)
    return z[:, HALO : HALO + ncols]


def _transient_patch(v, Ms, NKs, t1, t2, t3):
    """Additive correction for the first T0 columns: exact time-varying
    recursion minus the steady-state FIR result."""
    z = np.zeros(L, F32)
    z_exact = np.zeros((L, T0), F32)
    for t in range(T0):
        Mt = (Ms[t] if t < NRIC else Ms[-1]).astype(F32)
        NKt = (NKs[t] if t < NRIC else NKs[-1]).astype(F32)
        z = Mt @ z + NKt @ v[:, t]
        z_exact[:, t] = z
    vpad = np.concatenate([np.zeros((NV, HALO), F32), v[:, :T0]], axis=1)
    z_ss = _fir_host(vpad, t1, t2, t3, T0)
    return z_exact - z_ss


# ----------------------------------------------------------------------------
# device kernel
# ----------------------------------------------------------------------------

_CACHE = {}


def _build_nc():
    import concourse.mybir as mybir
    from concourse import bacc
    from concourse.tile import TileContext

    f32 = mybir.dt.float32
    # Bacc (not raw Bass): its compile() pipeline legalizes semaphore waits
    # (each TRN2 instruction supports at most one sync wait).
    nc = bacc.Bacc()

    v_d = nc.dram_tensor("v", [NV, WIDTH], f32, kind="ExternalInput")
    w1_d = nc.dram_tensor("w1", [NV, NTAP, L], f32, kind="ExternalInput")
    w2_d = nc.dram_tensor("w2", [L, NTAP, L], f32, kind="ExternalInput")
    w3_d = nc.dram_tensor("w3", [L, NTAP, L], f32, kind="ExternalInput")
    z_d = nc.dram_tensor("z", [L, TC], f32, kind="ExternalOutput")

    # output-column chunks per stage (global column index in [0, WIDTH)).
    # stage s computes cols [start, WIDTH); tap i reads src cols shifted by
    # -stride*i, so start must leave room: 48/60/64 for strides 16/4/1.
    chunks = lambda start: [
        (s, min(512, WIDTH - s)) for s in range(start, WIDTH, 512)
    ]

    with TileContext(nc) as tc:
        with (
            tc.tile_pool(name="sb", bufs=1) as sb,
            tc.tile_pool(name="psum", bufs=2, space="PSUM") as ps,
        ):
            v_sb = sb.tile([NV, WIDTH], f32)
            nc.sync.dma_start(out=v_sb, in_=v_d[:, :])
            w1_sb = sb.tile([NV, NTAP, L], f32)
            nc.sync.dma_start(out=w1_sb, in_=w1_d[:, :, :])
            w2_sb = sb.tile([L, NTAP, L], f32)
            nc.sync.dma_start(out=w2_sb, in_=w2_d[:, :, :])
            w3_sb = sb.tile([L, NTAP, L], f32)
            nc.sync.dma_start(out=w3_sb, in_=w3_d[:, :, :])

            def stage(src_sb, w_sb, stride, start_col, dst_sb):
                # one PSUM accumulator spanning all chunks; slice per matmul
                # (each matmul slice stays within one 2KB bank: 512-col steps).
                ncols = WIDTH - start_col
                acc = ps.tile([L, ncols], f32)
                for (s, w) in chunks(start_col):
                    for i in range(NTAP):
                        nc.tensor.matmul(
                            out=acc[:, s - start_col : s - start_col + w],
                            lhsT=w_sb[:, i, :],
                            rhs=src_sb[:, s - stride * i : s - stride * i + w],
                            start=(i == 0),
                            stop=(i == NTAP - 1),
                        )
                nc.vector.tensor_copy(out=dst_sb[:, start_col:WIDTH], in_=acc[:, :])

            g_sb = sb.tile([L, WIDTH], f32)
            h_sb = sb.tile([L, WIDTH], f32)
            z_sb = sb.tile([L, WIDTH], f32)
            stage(v_sb, w1_sb, 16, 48, g_sb)   # g valid from col 48
            stage(g_sb, w2_sb, 4, 60, h_sb)    # h valid from col 60
            stage(h_sb, w3_sb, 1, 64, z_sb)    # z valid from col 64
            nc.sync.dma_start(out=z_d[:, :], in_=z_sb[:, HALO:WIDTH])

    nc.compile()
    return nc


def kernel(inputs, controls, A, B, C, Q, R):
    from concourse.bass_utils import run_bass_kernel_spmd

    v = np.concatenate(
        [np.ascontiguousarray(controls, F32), np.ascontiguousarray(inputs, F32)],
        axis=0,
    )  # [NV, T], rows 0-63 = u, 64-127 = x

    Ms, NKs = _gains(A, B, C, Q, R)
    t1, t2, t3 = _taps(Ms[-1], NKs[-1])
    patch = _transient_patch(v, Ms, NKs, t1, t2, t3)

    # lhsT layouts: [K, tap, M] = tap matrix transposed
    w1 = np.ascontiguousarray(t1.transpose(2, 0, 1))  # [NV, NTAP, L]
    w2 = np.ascontiguousarray(t2.transpose(2, 0, 1))  # [L, NTAP, L]
    w3 = np.ascontiguousarray(t3.transpose(2, 0, 1))

    vpad = np.concatenate([np.zeros((NV, HALO), F32), v], axis=1)
    in_maps = [
        {
            "v": np.ascontiguousarray(vpad[:, i * TC : i * TC + WIDTH]),
            "w1": w1,
            "w2": w2,
            "w3": w3,
        }
        for i in range(NCORES)
    ]

    if "nc" not in _CACHE:
        _CACHE["nc"] = _build_nc()
    res = run_bass_kernel_spmd(_CACHE["nc"], in_maps, core_ids=list(range(NCORES)))

    z = np.concatenate([res.results[i]["z"] for i in range(NCORES)], axis=1)
    z[:, :T0] += patch
    return z



# revision 3
# speedup vs baseline: 2.4977x; 2.4977x over previous
"""Kalman filter kernel for 8x Trainium2 NeuronCores.

Math: the covariance/gain recursion (P_t, K_t) is data-independent and
converges to steady state within ~30 steps.  After convergence the state
recursion is the LTI scan  z_t = M z_{t-1} + NK @ [u_t; x_t]  with
M = (I-KC)A (spectral radius ~0.50),  NK = [(I-KC)B, K].  ||M^8|| ~ 3e-3,
so against the 2e-2 gate the scan truncates to an 8-tap causal FIR,
factored as two stages:

    g(t) = NK v(t) + M^4 NK v(t-4)            (2 taps, dilation 4, K=128)
    z(t) = sum_{r<4} M^r g(t-r)               (4 taps, dilation 1)

Stage 2's four K=64 taps are packed into two K=128 matmuls by stacking
[g(t); g(t-1)] on partitions (two shifted copies of stage 1's PSUM
output), so each core runs just 8 bf16 matmuls over its 1024 columns.
All matmuls are bf16 (fp32 runs 2-pass LOW/HIGH at 1/4 rate); the output
DMAs straight from PSUM.  Host adds two fp32 corrections: the transient
patch for t<96 (time-varying gains) and the 3 leading columns of cores
1..7 (left-halo taps the device reads as zeros).
"""

import numpy as np
import ml_dtypes

L = 64          # latent size
NV = 128        # stacked input dim [u; x]
T = 8192
NCORES = 8
TC = T // NCORES            # 1024 output columns per core
HALO = 8                    # left v-halo per core (stage1 reads back 4+stage2 3)
WIDTH = HALO + TC           # per-core input columns (1032)
S1TAPS = 2                  # stage-1 taps, dilation 4
S1DIL = 4
S2TAPS = 4                  # stage-2 taps, dilation 1 (packed 2x K=128)
NRIC = 64                   # Riccati iterations (converged far past f32 by then)
T0 = 96                     # transient patch columns
CHUNK = 512                 # PSUM bank = 512 fp32 columns

F32 = np.float32
BF16 = ml_dtypes.bfloat16


# ----------------------------------------------------------------------------
# host-side parameter preprocessing (data-independent)
# ----------------------------------------------------------------------------

def _gains(A, B, C, Q, R):
    """float64 Riccati recursion -> per-step (M_t, NK_t) lists."""
    Ad, Bd, Cd, Qd, Rd = (m.astype(np.float64) for m in (A, B, C, Q, R))
    P = np.eye(L)
    Ms, NKs = [], []
    for _ in range(NRIC):
        Pp = Ad @ P @ Ad.T + Qd
        S = Cd @ Pp @ Cd.T + Rd
        K = Pp @ Cd.T @ np.linalg.inv(S)
        P = Pp - K @ (Cd @ Pp)
        IKC = np.eye(L) - K @ Cd
        Ms.append(IKC @ Ad)
        NKs.append(np.concatenate([IKC @ Bd, K], axis=1))   # [L, NV]
    return Ms, NKs


def _mpow(M, k):
    out = np.eye(L)
    for _ in range(k):
        out = M @ out
    return out


def _bf(x):
    return np.asarray(x, F32).astype(BF16).astype(F32)


def _weights(Mss, NKss):
    """bf16 tap matrices.  w1[p] = M^(4p) NK  (stage 1, [L,NV]);
    w2[r] = M^r (stage 2, [L,L]).  Returned as f32 arrays holding exact
    bf16 values (shared by device upload and host replica)."""
    w1 = [_bf(_mpow(Mss, S1DIL * p) @ NKss) for p in range(S1TAPS)]
    w2 = [_bf(_mpow(Mss, r)) for r in range(S2TAPS)]
    return w1, w2


def _stage1_host(w1, vq, cols):
    """g at the given global columns (list), replicating the device:
    bf16 inputs/weights, fp32 accumulate.  vq: f32-holding-bf16 [NV,T]."""
    out = np.zeros((L, len(cols)), F32)
    for j, c in enumerate(cols):
        acc = np.zeros(L, F32)
        for p in range(S1TAPS):
            cc = c - S1DIL * p
            if cc >= 0:
                acc += w1[p] @ vq[:, cc]
        out[:, j] = acc
    return out


def _fir_host(w1, w2, vq, ncols):
    """Device-pipeline replica for global cols [0, ncols): zero left pad,
    bf16 rounding of g between stages."""
    pad = S1DIL * (S1TAPS - 1) + S2TAPS  # enough left context
    vp = np.concatenate([np.zeros((NV, pad), F32), vq[:, :ncols]], axis=1)
    n = vp.shape[1]
    g = np.zeros((L, n), F32)
    for p in range(S1TAPS):
        sh = S1DIL * p
        g[:, sh:] += (w1[p] @ vp[:, : n - sh]).astype(F32)
    gq = _bf(g)
    gq[:, :pad] = 0.0  # device sees zeros left of its first column
    z = np.zeros((L, n), F32)
    for r in range(S2TAPS):
        z[:, r:] += (w2[r] @ gq[:, : n - r]).astype(F32)
    return z[:, pad:]


def _transient_patch(v, vq, Ms, NKs, w1, w2):
    """Additive correction for cols [0,T0): exact time-varying recursion
    minus the device FIR replica."""
    z = np.zeros(L, F32)
    z_exact = np.zeros((L, T0), F32)
    for t in range(T0):
        Mt = (Ms[t] if t < NRIC else Ms[-1]).astype(F32)
        NKt = (NKs[t] if t < NRIC else NKs[-1]).astype(F32)
        z = Mt @ z + NKt @ v[:, t]
        z_exact[:, t] = z
    return z_exact - _fir_host(w1, w2, vq, T0)


# ----------------------------------------------------------------------------
# device kernel
# ----------------------------------------------------------------------------

_CACHE = {}


def _build_nc():
    import concourse.mybir as mybir
    from concourse import bacc
    from concourse.tile import TileContext

    f32 = mybir.dt.float32
    bf16 = mybir.dt.bfloat16
    nc = bacc.Bacc()

    v_d = nc.dram_tensor("v", [NV, WIDTH], bf16, kind="ExternalInput")
    w_d = nc.dram_tensor("w", [NV, S1TAPS + 2, L], bf16, kind="ExternalInput")
    z_d = nc.dram_tensor("z", [L, TC], f32, kind="ExternalOutput")

    chunks = [(HALO + i * CHUNK, HALO + (i + 1) * CHUNK) for i in range(TC // CHUNK)]

    with TileContext(nc) as tc:
        with (
            tc.tile_pool(name="sb", bufs=1) as sb,
            tc.tile_pool(name="ps1", bufs=2, space="PSUM") as ps1,
            tc.tile_pool(name="ps2", bufs=2, space="PSUM") as ps2,
        ):
            w_sb = sb.tile([NV, S1TAPS + 2, L], bf16)
            nc.sync.dma_start(out=w_sb, in_=w_d[:, :, :])
            v_sb = sb.tile([NV, WIDTH], bf16)
            for lo, hi in chunks:  # split so matmuls start after first chunk
                nc.sync.dma_start(
                    out=v_sb[:, lo - HALO : hi], in_=v_d[:, lo - HALO : hi]
                )

            # stacked stage-1 output: partitions 0-63 g(t), 64-127 g(t-1)
            gs = sb.tile([NV, WIDTH + 2], bf16)
            nc.vector.memset(gs[:, 0 : HALO + 1], 0.0)

            for ci, (lo, hi) in enumerate(chunks):
                acc = ps1.tile([L, CHUNK], f32)
                for p in range(S1TAPS):
                    nc.tensor.matmul(
                        out=acc,
                        lhsT=w_sb[:, p, :],
                        rhs=v_sb[:, lo - S1DIL * p : hi - S1DIL * p],
                        start=(p == 0),
                        stop=(p == S1TAPS - 1),
                    )
                nc.vector.tensor_copy(out=gs[0:L, lo:hi], in_=acc)
                nc.scalar.copy(out=gs[L:NV, lo + 1 : hi + 1], in_=acc)

            z_sb = sb.tile([L, TC], f32)
            for ci, (lo, hi) in enumerate(chunks):
                acc = ps2.tile([L, CHUNK], f32)
                nc.tensor.matmul(
                    out=acc, lhsT=w_sb[:, S1TAPS, :], rhs=gs[:, lo:hi],
                    start=True, stop=False,
                )
                nc.tensor.matmul(
                    out=acc, lhsT=w_sb[:, S1TAPS + 1, :], rhs=gs[:, lo - 2 : hi - 2],
                    start=False, stop=True,
                )
                nc.vector.tensor_copy(out=z_sb[:, lo - HALO : hi - HALO], in_=acc)
                nc.sync.dma_start(
                    out=z_d[:, lo - HALO : hi - HALO],
                    in_=z_sb[:, lo - HALO : hi - HALO],
                )

    nc.compile()
    return nc


def _prep(inputs, controls, A, B, C, Q, R):
    """Host preprocessing shared by kernel() and the profiling path.
    Returns (in_maps, patch, bfixes) where patch is the [L,T0] transient
    correction and bfixes[i] the [L,3] left-halo fix for core i>=1."""
    v = np.concatenate(
        [np.ascontiguousarray(controls, F32), np.ascontiguousarray(inputs, F32)],
        axis=0,
    )  # [NV, T]
    vq = _bf(v)

    Ms, NKs = _gains(A, B, C, Q, R)
    w1, w2 = _weights(Ms[-1], NKs[-1])
    patch = _transient_patch(v, vq, Ms, NKs, w1, w2)

    # device weight tensor: lhsT layout [K, slot, M]
    wslots = [w.T for w in w1]  # [NV, L] each
    wslots.append(np.concatenate([w2[0].T, w2[1].T], axis=0))  # [NV, L]
    wslots.append(np.concatenate([w2[2].T, w2[3].T], axis=0))
    w_dev = np.ascontiguousarray(
        np.stack(wslots, axis=1).astype(BF16)
    )  # [NV, 4, L]

    vpad = np.concatenate([np.zeros((NV, HALO), F32), vq], axis=1)
    in_maps = [
        {
            "v": np.ascontiguousarray(vpad[:, i * TC : i * TC + WIDTH]).astype(BF16),
            "w": w_dev,
        }
        for i in range(NCORES)
    ]

    # left-halo fixes: device g is zero for local cols < HALO, i.e. global
    # cols < i*TC; output col j in {0,1,2} of core i>=1 is missing
    # sum_{r>j} w2[r] g(i*TC + j - r).
    bfixes = {}
    for i in range(1, NCORES):
        gcols = [i * TC - 3, i * TC - 2, i * TC - 1]
        gh = _bf(_stage1_host(w1, vq, gcols))  # [L,3] bf16-rounded like device
        fix = np.zeros((L, 3), F32)
        for j in range(3):
            for r in range(j + 1, S2TAPS):
                fix[:, j] += w2[r] @ gh[:, 3 + j - r]
        bfixes[i] = fix
    return in_maps, patch, bfixes


def kernel(inputs, controls, A, B, C, Q, R):
    from concourse.bass_utils import run_bass_kernel_spmd

    in_maps, patch, bfixes = _prep(inputs, controls, A, B, C, Q, R)

    if "nc" not in _CACHE:
        _CACHE["nc"] = _build_nc()
    res = run_bass_kernel_spmd(_CACHE["nc"], in_maps, core_ids=list(range(NCORES)))

    z = np.concatenate(
        [np.asarray(res.results[i]["z"], F32) for i in range(NCORES)], axis=1
    )
    z[:, :T0] += patch
    for i, fix in bfixes.items():
        z[:, i * TC : i * TC + 3] += fix
    return z


# revision 5
# speedup vs baseline: 2.7457x; 1.0993x over previous
"""Kalman filter kernel for 8x Trainium2 NeuronCores.

Math: the covariance/gain recursion (P_t, K_t) is data-independent and
converges to steady state within ~30 steps.  After convergence the state
recursion is the LTI scan  z_t = M z_{t-1} + NK @ [u_t; x_t]  with
M = (I-KC)A (spectral radius ~0.50),  NK = [(I-KC)B, K].  ||M^8|| ~ 3e-3,
so against the 2e-2 gate the scan truncates to an 8-tap causal FIR,
factored as two stages:

    g(t) = NK v(t) + M^4 NK v(t-4)            (2 taps, dilation 4, K=128)
    z(t) = sum_{r<4} M^r g(t-r)               (4 taps, dilation 1)

Stage 2's four K=64 taps are packed into two K=128 matmuls by stacking
[g(t); g(t-1)] on partitions (two shifted copies of stage 1's PSUM
output), so each core runs just 8 bf16 matmuls over its 1024 columns.
All matmuls are bf16 (fp32 runs 2-pass LOW/HIGH at 1/4 rate); the output
DMAs straight from PSUM.  Host adds two fp32 corrections: the transient
patch for t<96 (time-varying gains) and the 3 leading columns of cores
1..7 (left-halo taps the device reads as zeros).
"""

import numpy as np
import ml_dtypes

L = 64          # latent size
NV = 128        # stacked input dim [u; x]
T = 8192
NCORES = 8
TC = T // NCORES            # 1024 output columns per core
HALO = 8                    # left v-halo per core (stage1 reads back 4+stage2 3)
WIDTH = HALO + TC           # per-core input columns (1032)
S1TAPS = 2                  # stage-1 taps, dilation 4
S1DIL = 4
S2TAPS = 4                  # stage-2 taps, dilation 1 (packed 2x K=128)
NRIC = 64                   # Riccati iterations (converged far past f32 by then)
T0 = 96                     # transient patch columns
CHUNK = 512                 # PSUM bank = 512 fp32 columns

F32 = np.float32
BF16 = ml_dtypes.bfloat16


# ----------------------------------------------------------------------------
# host-side parameter preprocessing (data-independent)
# ----------------------------------------------------------------------------

def _gains(A, B, C, Q, R):
    """float64 Riccati recursion -> per-step (M_t, NK_t) lists."""
    Ad, Bd, Cd, Qd, Rd = (m.astype(np.float64) for m in (A, B, C, Q, R))
    P = np.eye(L)
    Ms, NKs = [], []
    for _ in range(NRIC):
        Pp = Ad @ P @ Ad.T + Qd
        S = Cd @ Pp @ Cd.T + Rd
        K = Pp @ Cd.T @ np.linalg.inv(S)
        P = Pp - K @ (Cd @ Pp)
        IKC = np.eye(L) - K @ Cd
        Ms.append(IKC @ Ad)
        NKs.append(np.concatenate([IKC @ Bd, K], axis=1))   # [L, NV]
    return Ms, NKs


def _mpow(M, k):
    out = np.eye(L)
    for _ in range(k):
        out = M @ out
    return out


def _bf(x):
    return np.asarray(x, F32).astype(BF16).astype(F32)


def _weights(Mss, NKss):
    """bf16 tap matrices.  w1[p] = M^(4p) NK  (stage 1, [L,NV]);
    w2[r] = M^r (stage 2, [L,L]).  Returned as f32 arrays holding exact
    bf16 values (shared by device upload and host replica)."""
    w1 = [_bf(_mpow(Mss, S1DIL * p) @ NKss) for p in range(S1TAPS)]
    w2 = [_bf(_mpow(Mss, r)) for r in range(S2TAPS)]
    return w1, w2


def _stage1_host(w1, vq, cols):
    """g at the given global columns (list), replicating the device:
    bf16 inputs/weights, fp32 accumulate.  vq: f32-holding-bf16 [NV,T]."""
    out = np.zeros((L, len(cols)), F32)
    for j, c in enumerate(cols):
        acc = np.zeros(L, F32)
        for p in range(S1TAPS):
            cc = c - S1DIL * p
            if cc >= 0:
                acc += w1[p] @ vq[:, cc]
        out[:, j] = acc
    return out


def _fir_host(w1, w2, vq, ncols):
    """Device-pipeline replica for global cols [0, ncols): zero left pad,
    bf16 rounding of g between stages."""
    pad = S1DIL * (S1TAPS - 1) + S2TAPS  # enough left context
    vp = np.concatenate([np.zeros((NV, pad), F32), vq[:, :ncols]], axis=1)
    n = vp.shape[1]
    g = np.zeros((L, n), F32)
    for p in range(S1TAPS):
        sh = S1DIL * p
        g[:, sh:] += (w1[p] @ vp[:, : n - sh]).astype(F32)
    gq = _bf(g)
    gq[:, :pad] = 0.0  # device sees zeros left of its first column
    z = np.zeros((L, n), F32)
    for r in range(S2TAPS):
        z[:, r:] += (w2[r] @ gq[:, : n - r]).astype(F32)
    return z[:, pad:]


def _transient_patch(v, vq, Ms, NKs, w1, w2):
    """Additive correction for cols [0,T0): exact time-varying recursion
    minus the device FIR replica."""
    z = np.zeros(L, F32)
    z_exact = np.zeros((L, T0), F32)
    for t in range(T0):
        Mt = (Ms[t] if t < NRIC else Ms[-1]).astype(F32)
        NKt = (NKs[t] if t < NRIC else NKs[-1]).astype(F32)
        z = Mt @ z + NKt @ v[:, t]
        z_exact[:, t] = z
    return z_exact - _fir_host(w1, w2, vq, T0)


# ----------------------------------------------------------------------------
# device kernel
# ----------------------------------------------------------------------------

_CACHE = {}


def _build_nc():
    import concourse.mybir as mybir
    from concourse import bacc
    from concourse.tile import TileContext

    f32 = mybir.dt.float32
    bf16 = mybir.dt.bfloat16
    nc = bacc.Bacc()

    v_d = nc.dram_tensor("v", [NV, WIDTH], bf16, kind="ExternalInput")
    w_d = nc.dram_tensor("w", [NV, S1TAPS + 2, L], bf16, kind="ExternalInput")
    z_d = nc.dram_tensor("z", [L, TC], bf16, kind="ExternalOutput")

    chunks = [(HALO + i * CHUNK, HALO + (i + 1) * CHUNK) for i in range(TC // CHUNK)]
    NWARM = 6

    with TileContext(nc) as tc:
        with (
            tc.tile_pool(name="sb", bufs=1) as sb,
            tc.tile_pool(name="ps1", bufs=2, space="PSUM") as ps1,
            tc.tile_pool(name="ps2", bufs=2, space="PSUM") as ps2,
            tc.tile_pool(name="psw", bufs=1, space="PSUM") as psw,
        ):
            # input DMAs spread over the three DMA queues (sync-HW,
            # scalar-HW, gpsimd-SW) -- a single queue moves only ~25-44
            # GB/s, and it is the critical path before the first matmul.
            w_sb = sb.tile([NV, S1TAPS + 2, L], bf16)
            v_sb = sb.tile([NV, WIDTH], bf16)
            nc.sync.dma_start(out=w_sb[:, 0:2, :], in_=w_d[:, 0:2, :])
            nc.scalar.dma_start(out=w_sb[:, 2:4, :], in_=w_d[:, 2:4, :])
            nc.gpsimd.dma_start(out=v_sb[:, 520:WIDTH], in_=v_d[:, 520:WIDTH])
            nc.sync.dma_start(out=v_sb[:, 0:264], in_=v_d[:, 0:264])
            nc.scalar.dma_start(out=v_sb[:, 264:520], in_=v_d[:, 264:520])

            # PE p-state ramps to 2.4 GHz only after ~3us of continuous
            # work; burn the input-DMA wait warming it on a zeroed tile.
            scratch = sb.tile([NV, CHUNK], bf16)
            nc.vector.memset(scratch, 0.0)
            wacc = psw.tile([L, CHUNK], f32)
            for _ in range(NWARM):
                nc.tensor.matmul(
                    out=wacc, lhsT=scratch[:, 0:L], rhs=scratch,
                    start=True, stop=True,
                )

            # stacked stage-1 output: partitions 0-63 g(t), 64-127 g(t-1)
            gs = sb.tile([NV, WIDTH + 2], bf16)
            nc.vector.memset(gs[0:L, 0:HALO], 0.0)
            nc.vector.memset(gs[L:NV, 0 : HALO + 1], 0.0)

            z_sb = sb.tile([L, TC], bf16)
            for ci, (lo, hi) in enumerate(chunks):
                acc = ps1.tile([L, CHUNK], f32)
                for p in range(S1TAPS):
                    nc.tensor.matmul(
                        out=acc,
                        lhsT=w_sb[:, p, :],
                        rhs=v_sb[:, lo - S1DIL * p : hi - S1DIL * p],
                        start=(p == 0),
                        stop=(p == S1TAPS - 1),
                    )
                nc.vector.tensor_copy(out=gs[0:L, lo:hi], in_=acc)
                nc.vector.tensor_copy(out=gs[L:NV, lo + 1 : hi + 1], in_=acc)

                acc2 = ps2.tile([L, CHUNK], f32)
                nc.tensor.matmul(
                    out=acc2, lhsT=w_sb[:, S1TAPS, :], rhs=gs[:, lo:hi],
                    start=True, stop=False,
                )
                nc.tensor.matmul(
                    out=acc2, lhsT=w_sb[:, S1TAPS + 1, :], rhs=gs[:, lo - 2 : hi - 2],
                    start=False, stop=True,
                )
                nc.vector.tensor_copy(out=z_sb[:, lo - HALO : hi - HALO], in_=acc2)
                dma_eng = nc.sync if ci == 0 else nc.scalar
                dma_eng.dma_start(
                    out=z_d[:, lo - HALO : hi - HALO],
                    in_=z_sb[:, lo - HALO : hi - HALO],
                )

    nc.compile()
    return nc


def _prep(inputs, controls, A, B, C, Q, R):
    """Host preprocessing shared by kernel() and the profiling path.
    Returns (in_maps, patch, bfixes) where patch is the [L,T0] transient
    correction and bfixes[i] the [L,3] left-halo fix for core i>=1."""
    v = np.concatenate(
        [np.ascontiguousarray(controls, F32), np.ascontiguousarray(inputs, F32)],
        axis=0,
    )  # [NV, T]
    vq = _bf(v)

    Ms, NKs = _gains(A, B, C, Q, R)
    w1, w2 = _weights(Ms[-1], NKs[-1])
    patch = _transient_patch(v, vq, Ms, NKs, w1, w2)

    # device weight tensor: lhsT layout [K, slot, M]
    wslots = [w.T for w in w1]  # [NV, L] each
    wslots.append(np.concatenate([w2[0].T, w2[1].T], axis=0))  # [NV, L]
    wslots.append(np.concatenate([w2[2].T, w2[3].T], axis=0))
    w_dev = np.ascontiguousarray(
        np.stack(wslots, axis=1).astype(BF16)
    )  # [NV, 4, L]

    vpad = np.concatenate([np.zeros((NV, HALO), F32), vq], axis=1)
    in_maps = [
        {
            "v": np.ascontiguousarray(vpad[:, i * TC : i * TC + WIDTH]).astype(BF16),
            "w": w_dev,
        }
        for i in range(NCORES)
    ]

    # left-halo fixes: device g is zero for local cols < HALO, i.e. global
    # cols < i*TC; output col j in {0,1,2} of core i>=1 is missing
    # sum_{r>j} w2[r] g(i*TC + j - r).
    bfixes = {}
    for i in range(1, NCORES):
        gcols = [i * TC - 3, i * TC - 2, i * TC - 1]
        gh = _bf(_stage1_host(w1, vq, gcols))  # [L,3] bf16-rounded like device
        fix = np.zeros((L, 3), F32)
        for j in range(3):
            for r in range(j + 1, S2TAPS):
                fix[:, j] += w2[r] @ gh[:, 3 + j - r]
        bfixes[i] = fix
    return in_maps, patch, bfixes


def kernel(inputs, controls, A, B, C, Q, R):
    from concourse.bass_utils import run_bass_kernel_spmd

    in_maps, patch, bfixes = _prep(inputs, controls, A, B, C, Q, R)

    if "nc" not in _CACHE:
        _CACHE["nc"] = _build_nc()
    res = run_bass_kernel_spmd(_CACHE["nc"], in_maps, core_ids=list(range(NCORES)))

    z = np.concatenate(
        [np.asarray(res.results[i]["z"]).astype(F32) for i in range(NCORES)], axis=1
    )
    z[:, :T0] += patch
    for i, fix in bfixes.items():
        z[:, i * TC : i * TC + 3] += fix
    return z
